# revision 15
# baseline (speedup 1.0000x reference)
"""Trainium2 Bass kernel v2: 3-layer SAGE+GCN GNN, 50k nodes / 800k edges,
8 NeuronCores.

Key structure (vs v1):
  - Node rows are stored core-padded: node g -> table row (g//shard)*ntP +
    g%shard, ntP = nt*128. Regions of 25088 rows keep gather idx in int16.
  - Gathers are batched: one dma_gather per (edge-set, region, tile-group)
    (GT tiles/group), amortizing the ~1us SWDGE fixed cost.
  - Gather idx + M-build scalars (dl, w) live in SBUF persistently, loaded
    once and reused by all 3 layers (no per-tile idx DMAs).
  - lin_r (SAGE root term) is folded into the aggregation PSUM accumulation
    as extra matmuls; epilogues use fused scalar_tensor_tensor ops.
  - x0/x1a/x1b residual tensors are bf16 (halves SBUF, kills bf16 copies).
  - Layer-1 table is built from the own shard only and AllGathered (same as
    layers 2/3); no replicated full-table build, no full-x upload.
  - M-builds are split between DVE and Pool engines for balance.
"""

import os
import numpy as np
import ml_dtypes

P = 128
NCORES = 8

LAST_EXEC_NS = None
LAST_TRACE = None

N = 50000
SHARD = N // NCORES            # 6250
NT = (SHARD + P - 1) // P      # 49
NTP = NT * P                   # 6272
NROWS = NCORES * NTP           # 50176
REG = 4 * NTP                  # 25088 rows per gather region (int16-safe)
NREG = 2
GT = int(os.environ.get("GNN_GT", "4"))          # tiles per gather group
NGRP = (NT + GT - 1) // GT
MAXROWS = int(os.environ.get("GNN_GROWS", "1024"))  # max rows per dma_gather
POOL_FRAC = float(os.environ.get("GNN_POOLF", "0"))  # M-builds on Pool
NQUEUES = int(os.environ.get("GNN_NQ", "4"))        # SWDGE queues (1-4)


# ----------------------------------------------------------------------------
# host-side preprocessing
# ----------------------------------------------------------------------------

def _edge_streams(srcrow, dst, w_edge):
    """Per-core gather streams, grouped per (region r, tile-group g).

    Layout per core:
      chunks are ordered call-major: for r in 0..1: for g in 0..NGRP-1:
        for t in group(g): K[t][r] chunks of 128 slots.
      slot j within a call -> partition j%128, call-chunk j//128.
    Returns per-core (idx16 [128, ICOLS], dw [128, 2*SK]) + layout meta.
    """
    percore = []
    counts = np.zeros((NCORES, NT, NREG), np.int64)
    for c in range(NCORES):
        lo, hi = c * SHARD, (c + 1) * SHARD
        m = (dst >= lo) & (dst < hi)
        s_c, d_c, w_c = srcrow[m], dst[m] - lo, w_edge[m]
        t_c = d_c >> 7
        r_c = s_c // REG
        order = np.lexsort((t_c, r_c))      # by (region, tile); grp implied
        s_c, d_c, w_c = s_c[order], d_c[order], w_c[order]
        t_c, r_c = t_c[order], r_c[order]
        key = r_c * NT + t_c
        bounds = np.searchsorted(key, np.arange(NREG * NT + 1))
        counts[c] = (bounds[1:] - bounds[:-1]).reshape(NREG, NT).T
        percore.append((s_c, d_c, w_c, bounds))
    K = np.maximum((counts.max(axis=0) + P - 1) // P, 1)   # [NT, NREG]

    # chunk-column layout, call-major
    groups = [list(range(g * GT, min((g + 1) * GT, NT))) for g in range(NGRP)]
    cof = np.zeros((NT, NREG), np.int64)    # chunk col of (t, r) within set
    callinfo = []                           # (r, g, colbase, nchunks, icol)
    acc = 0
    icol = 0
    for r in range(NREG):
        for g, tl in enumerate(groups):
            base = acc
            for t in tl:
                cof[t, r] = acc
                acc += K[t, r]
            nch = acc - base
            callinfo.append((r, g, base, nch, icol))
            icol += nch * 8
    SK = int(acc)
    ICOLS = int(icol)

    outs = []
    for c in range(NCORES):
        s_c, d_c, w_c, bounds = percore[c]
        idx16 = np.zeros((P, ICOLS), np.int16)
        dl = np.full((P, SK), -1.0, np.float32)
        wv = np.zeros((P, SK), np.float32)
        for (r, g, base, nch, ic) in callinfo:
            slots = nch * P
            buf_i = np.zeros(slots, np.int32)
            buf_d = np.full(slots, -1.0, np.float32)
            buf_w = np.zeros(slots, np.float32)
            pos = 0
            for t in groups[g]:
                b0, b1 = bounds[r * NT + t], bounds[r * NT + t + 1]
                cnt = b1 - b0
                kr = int(K[t, r])
                buf_i[pos:pos + cnt] = s_c[b0:b1] - r * REG
                buf_d[pos:pos + cnt] = (d_c[b0:b1] - t * P).astype(np.float32)
                buf_w[pos:pos + cnt] = w_c[b0:b1]
                pos += kr * P
            assert pos == slots
            cols = nch * 8
            wrap = buf_i.reshape(cols * 16 // 16, 16).T.astype(np.int16)
            idx16[:, ic:ic + cols] = np.tile(wrap, (8, 1))
            dl[:, base:base + nch] = buf_d.reshape(nch, P).T
            wv[:, base:base + nch] = buf_w.reshape(nch, P).T
        dw = np.concatenate([dl, wv], axis=1).astype(np.float32)
        outs.append((idx16, dw))
    meta_es = dict(K=K.tolist(), cof=cof.tolist(), SK=SK, ICOLS=ICOLS,
                   callinfo=callinfo, groups=groups)
    return outs, meta_es


def _prep(inputs):
    inp = {k: np.asarray(v) for k, v in inputs.items()}
    x = inp["x"].astype(np.float32)
    n, din = x.shape
    assert n == N and din == P

    src = inp["edge_index"][0].astype(np.int64)
    dst = inp["edge_index"][1].astype(np.int64)
    srca = inp["edge_index_aux"][0].astype(np.int64)
    dsta = inp["edge_index_aux"][1].astype(np.int64)

    deg = np.zeros(n, np.float32)
    np.add.at(deg, dst, 1.0)
    recip_deg = (1.0 / np.maximum(deg, 1.0)).astype(np.float32)
    dega = np.zeros(n, np.float32)
    np.add.at(dega, dsta, 1.0)
    deg_hat = dega + 1.0
    rs = (1.0 / np.sqrt(deg_hat)).astype(np.float32)

    rowmap = (src // SHARD) * NTP + (src % SHARD)
    sage_streams, es_s = _edge_streams(rowmap, dst, recip_deg[dst])
    # GCN self-loop folded as an (i,i) edge with w=rs[i] (table row carries
    # the other rs factor)
    allnodes = np.arange(n, dtype=np.int64)
    srca_x = np.concatenate([srca, allnodes])
    dsta_x = np.concatenate([dsta, allnodes])
    rowa = (srca_x // SHARD) * NTP + (srca_x % SHARD)
    gcn_streams, es_g = _edge_streams(rowa, dsta_x, rs[dsta_x])

    bf16 = ml_dtypes.bfloat16

    # packed bf16 weights [P, ncols]
    def w2(a):
        a = np.asarray(a, np.float32)
        return [a[i * P:(i + 1) * P] for i in range(a.shape[0] // P)]

    wb_tiles = []
    wb_off = {}

    def put_b(name, tiles):
        wb_off[name] = len(wb_tiles) * P
        wb_tiles.extend(tiles)

    put_b("fc1", w2(inp["fc1_W"]))
    for l in (1, 2, 3):
        put_b(f"sWl{l}", w2(inp[f"s{l}_Wl"]))
        put_b(f"gW{l}", w2(inp[f"g{l}_W"]))
        put_b(f"sWr{l}", w2(inp[f"s{l}_Wr"]))
    wb = np.concatenate(wb_tiles, axis=1).astype(bf16)

    # head columns (weight * w_i), bf16, padded per head to 2 cols of [P]
    w_scal = [float(inp[f"w{i}"][0]) for i in range(1, 5)]
    hb = np.zeros((P, 7), np.float32)
    hb_off = {}
    col = 0
    for i in range(1, 5):
        hw_ = inp[f"l{i}_W"].reshape(-1) * w_scal[i - 1]
        ncol = hw_.shape[0] // P
        hb_off[f"h{i}"] = col
        for j in range(ncol):
            hb[:, col] = hw_[j * P:(j + 1) * P]
            col += 1
    hb = hb.astype(bf16)

    # packed fp32 consts [P, ncols]
    wf_cols = []
    wf_off = {}

    def put_f(name, cols):
        a = np.asarray(cols, np.float32).reshape(-1)
        wf_off[name] = len(wf_cols)
        for i in range(a.shape[0] // P):
            wf_cols.append(a[i * P:(i + 1) * P])

    put_f("fc1_b", inp["fc1_b"])
    for l in (1, 2, 3):
        put_f(f"s_bl{l}", inp[f"s{l}_bl"])
        put_f(f"g_b{l}", inp[f"g{l}_b"])
    wf = np.stack(wf_cols, axis=1).astype(np.float32)
    total_bias = float(sum(float(inp[f"l{i}_b"][0]) * w_scal[i - 1]
                           for i in range(1, 5)))

    iota = np.broadcast_to(np.arange(P, dtype=np.float32), (P, P)).astype(bf16)
    iota = np.ascontiguousarray(iota)
    ident = np.eye(P, dtype=np.float32).astype(bf16)

    meta = dict(es_s=es_s, es_g=es_g, wb_off=wb_off, wf_off=wf_off,
                hb_off=hb_off, wb_cols=wb.shape[1], wf_cols=wf.shape[1],
                total_bias=total_bias)

    in_maps = []
    for c in range(NCORES):
        lo = c * SHARD
        nown = min(SHARD, n - lo)
        ownx = np.zeros((NTP, P), np.float32)
        ownx[:nown] = x[lo:lo + nown]
        # transposed own x tiles: xto[t*128+f, j] = x_own[t*128+j, f]
        xto = np.ascontiguousarray(
            ownx.reshape(NT, P, P).transpose(0, 2, 1).reshape(NTP, P)
        ).astype(bf16)
        rso = np.ones(NTP, np.float32)
        rso[:nown] = rs[lo:lo + nown]
        idx_s, dw_s = sage_streams[c]
        idx_g, dw_g = gcn_streams[c]
        in_maps.append({
            "xto": xto,
            "idxs": idx_s, "dws": dw_s,
            "idxg": idx_g, "dwg": dw_g,
            "wb": wb, "hb": hb, "wf": wf, "iota": iota, "ident": ident,
            "rso": rso.reshape(NT, P).T.copy(),
        })
    return meta, in_maps


# ----------------------------------------------------------------------------
# device program
# ----------------------------------------------------------------------------

def _build(meta):
    import concourse.bacc as bacc
    import concourse.mybir as mybir
    import concourse.tile as tile

    dt = mybir.dt
    Alu = mybir.AluOpType
    Act = mybir.ActivationFunctionType

    es_s, es_g = meta["es_s"], meta["es_g"]
    SKs, SKg = es_s["SK"], es_g["SK"]
    ICs, ICg = es_s["ICOLS"], es_g["ICOLS"]
    wbo, wfo, hbo = meta["wb_off"], meta["wf_off"], meta["hb_off"]
    groups = es_s["groups"]

    nc = bacc.Bacc("TRN2", target_bir_lowering=False, debug=False,
                   num_devices=NCORES, num_swdge_queues=NQUEUES)

    def din(name, shape, dtype):
        return nc.dram_tensor(name, shape, dtype, kind="ExternalInput")

    xto_d = din("xto", [NTP, P], dt.bfloat16)
    idxs_d = din("idxs", [P, ICs], dt.int16)
    dws_d = din("dws", [P, 2 * SKs], dt.float32)
    idxg_d = din("idxg", [P, ICg], dt.int16)
    dwg_d = din("dwg", [P, 2 * SKg], dt.float32)
    wb_d = din("wb", [P, meta["wb_cols"]], dt.bfloat16)
    hb_d = din("hb", [P, 7], dt.bfloat16)
    wf_d = din("wf", [P, meta["wf_cols"]], dt.float32)
    iota_d = din("iota", [P, P], dt.bfloat16)
    ident_d = din("ident", [P, P], dt.bfloat16)
    rso_d = din("rso", [P, NT], dt.float32)
    res_d = nc.dram_tensor("res", [P, NT], dt.float32, kind="ExternalOutput")

    with tile.TileContext(nc) as tc:
        import contextlib
        _stack = contextlib.ExitStack()
        _ppool = _stack.enter_context(tc.tile_pool(name="persist", bufs=1))
        _dpool = _stack.enter_context(
            tc.tile_pool(name="persistd", bufs=1, space="DRAM"))

        def tc_tile(shape, dtype, space="SBUF", addr_space="Local", name="t"):
            pool = _dpool if space == "DRAM" else _ppool
            return pool.tile(shape, dtype, tag=name, name=name,
                             addr_space=addr_space)

        f32, b16 = dt.float32, dt.bfloat16
        # --- persistent SBUF ---
        x0T = tc_tile([P, NTP], b16, name="x0T")
        x1aT = tc_tile([P, NTP], b16, name="x1aT")
        x1bT = tc_tile([P, NTP], b16, name="x1bT")
        lr3T = tc_tile([P, NTP], b16, name="lr3T")
        ident_s = tc_tile([P, P], b16, name="ident_s")
        resb = tc_tile([P, NT], f32, name="resb")
        wb_s = tc_tile([P, meta["wb_cols"]], b16, name="wb_s")
        hb_s = tc_tile([P, 7], b16, name="hb_s")
        wf_s = tc_tile([P, meta["wf_cols"]], f32, name="wf_s")
        iota_s = tc_tile([P, P], b16, name="iota_s")
        rso_s = tc_tile([P, NT], f32, name="rso_s")
        dws_s = tc_tile([P, 2 * SKs], f32, name="dws_s")
        dwg_s = tc_tile([P, 2 * SKg], f32, name="dwg_s")
        idxs_s = tc_tile([P, ICs], dt.int16, name="idxs_s")
        idxg_s = tc_tile([P, ICg], dt.int16, name="idxg_s")

        # --- DRAM tables ---
        tbls = [tc_tile([NROWS, 2 * P], b16, space="DRAM",
                        addr_space="Shared", name=f"tbl{i}")
                for i in (1, 2, 3)]
        shs = [tc_tile([NTP, 2 * P], b16, space="DRAM", name=f"sh{i}")
               for i in (1, 2, 3)]

        for t_, d_ in ((wb_s, wb_d), (hb_s, hb_d), (wf_s, wf_d),
                       (iota_s, iota_d), (ident_s, ident_d), (rso_s, rso_d),
                       (dws_s, dws_d), (dwg_s, dwg_d),
                       (idxs_s, idxs_d), (idxg_s, idxg_d)):
            nc.sync.dma_start(out=t_[:], in_=d_[:])

        with (
            tc.tile_pool(name="xp", bufs=3) as xp,
            tc.tile_pool(name="gp", bufs=int(os.environ.get("GNN_GB", "2"))) as gp,
            tc.tile_pool(name="mp", bufs=6) as mp,
            tc.tile_pool(name="op", bufs=6) as op,
            tc.tile_pool(name="tp", bufs=2) as tp,
            tc.tile_pool(name="pp", bufs=2, space="PSUM") as pp,
            tc.tile_pool(name="ph", bufs=2, space="PSUM") as php,
            tc.tile_pool(name="pq", bufs=2, space="PSUM") as pq,
        ):
            def wbt(name, half=0):
                o = wbo[name] + half * P
                return wb_s[:, o:o + P]

            def wfc(name, half=0):
                o = wfo[name] + half
                return wf_s[:, o:o + 1]

            def hcol(name, half=0):
                o = hbo[name] + half
                return hb_s[:, o:o + 1]

            mb_ctr = [0]
            q_ctr = [0]

            def mbuild(m_ap, dw, sk, col):
                # narrow fused one-hot*weight build; split DVE / Pool
                mb_ctr[0] += 1
                eng = (nc.gpsimd if (mb_ctr[0] % 10) < 10 * POOL_FRAC
                       else nc.vector)
                eng.tensor_scalar(m_ap, iota_s[:],
                                  dw[:, col:col + 1],
                                  dw[:, sk + col:sk + col + 1],
                                  op0=Alu.is_equal, op1=Alu.mult)

            def gathers_for(es, idx_s, tbl, colofs, r, g):
                """Issue dma_gather(s) for call (r, g); returns gbuf tile."""
                # find call info
                ci = None
                for (r_, g_, base, nch, ic) in es["callinfo"]:
                    if r_ == r and g_ == g:
                        ci = (base, nch, ic)
                        break
                base, nch, ic = ci
                gbuf = gp.tile([P, nch * P], b16, tag=f"g{colofs}{r}")
                rlo = r * REG
                rhi = min(NROWS, (r + 1) * REG)
                capk = MAXROWS // P
                qs = (0 if colofs == 0 else 2) + r
                for s in range(0, nch, capk):
                    kk = min(capk, nch - s)
                    q_ctr[0] += 1
                    qn = (qs if int(os.environ.get("GNN_QS", "0"))
                          else q_ctr[0])
                    nc.gpsimd.dma_gather(
                        out_ap=gbuf[:, s * P:(s + kk) * P]
                        .rearrange("p (k e) -> p k e", e=P),
                        in_ap=tbl[rlo:rhi, colofs:colofs + P],
                        idxs_ap=idx_s[:, ic + s * 8:ic + (s + kk) * 8],
                        num_idxs=kk * P,
                        num_idxs_reg=kk * P,
                        elem_size=P,
                        elem_step=2 * P,
                        queue_num=qn % NQUEUES)
                return gbuf, base

            # ---- layer-0 own pass: x0T, sh1, head1 ----
            for gi, tl in enumerate(groups):
                ntl = len(tl)
                t0 = tl[0]
                xo = xp.tile([P, ntl * P], b16, tag="xo")
                nc.sync.dma_start(
                    out=xo[:].rearrange("p (j c) -> p j c", c=P),
                    in_=xto_d[t0 * P:(t0 + ntl) * P, :]
                    .rearrange("(j p) c -> p j c", p=P))
                tbs = tp.tile([P, ntl * 2 * P], b16, tag="tbs")
                for j, t in enumerate(tl):
                    sl = slice(t * P, (t + 1) * P)
                    p1 = pq.tile([P, P], f32, tag="pa")
                    nc.tensor.matmul(p1[:], lhsT=wbt("fc1"),
                                     rhs=xo[:, j * P:(j + 1) * P],
                                     start=True, stop=True)
                    nc.scalar.activation(x0T[:, sl], p1[:], Act.Relu,
                                         bias=wfc("fc1_b"))
                    ps = pp.tile([P, P], f32, tag="ps")
                    nc.tensor.matmul(ps[:], lhsT=x0T[:, sl], rhs=wbt("sWl1"),
                                     start=True, stop=True)
                    pg = pp.tile([P, P], f32, tag="pg")
                    nc.tensor.matmul(pg[:], lhsT=x0T[:, sl], rhs=wbt("gW1"),
                                     start=True, stop=True)
                    nc.scalar.activation(tbs[:, (2 * j) * P:(2 * j + 1) * P],
                                         ps[:], Act.Copy)
                    nc.scalar.activation(tbs[:, (2 * j + 1) * P:(2 * j + 2) * P],
                                         pg[:], Act.Copy,
                                         scale=rso_s[:, t:t + 1])
                    phd = php.tile([P, 1], f32, tag="ph")
                    nc.tensor.matmul(phd[:], lhsT=x0T[:, sl], rhs=hcol("h1"),
                                     start=True, stop=True)
                    nc.vector.tensor_copy(resb[:, t:t + 1], phd[:])
                nc.sync.dma_start(
                    out=shs[0][t0 * P:(t0 + ntl) * P, :]
                    .rearrange("(j p) c -> p j c", p=P),
                    in_=tbs[:].rearrange("p (j c) -> p j c", c=2 * P))

            if not int(os.environ.get("GNN_NOBAR", "1")):
                tc.strict_bb_all_engine_barrier()
            nc.gpsimd.collective_compute(
                "AllGather", mybir.AluOpType.bypass,
                replica_groups=[list(range(NCORES))],
                ins=[shs[0][:]], outs=[tbls[0][:]])
            if not int(os.environ.get("GNN_NOBAR2", "0")):
                tc.strict_bb_all_engine_barrier()

            # ---- conv layers ----
            for l in (1, 2, 3):
                tbl = tbls[l - 1]
                sh_next = shs[l] if l < 3 else None
                for gi, tl in enumerate(groups):
                    ntl = len(tl)
                    t0 = tl[0]
                    bufs = {}
                    for kind, es, idx_s, colofs in (
                            ("s", es_s, idxs_s, 0), ("g", es_g, idxg_s, P)):
                        for r in range(NREG):
                            bufs[(kind, r)] = gathers_for(
                                es, idx_s, tbl[:], colofs, r, gi)
                    if l < 3:
                        tbs = tp.tile([P, ntl * 2 * P], b16, tag="tbs")
                    for j, t in enumerate(tl):
                        sl = slice(t * P, (t + 1) * P)
                        outs = {}
                        for kind, es, dw, sk in (("s", es_s, dws_s, SKs),
                                                 ("g", es_g, dwg_s, SKg)):
                            pa = pq.tile([P, P], f32, tag="pa")
                            ktot = sum(es["K"][t][r] for r in range(NREG))
                            first = True
                            kdone = 0
                            for r in range(NREG):
                                gbuf, base = bufs[(kind, r)]
                                kt = es["K"][t][r]
                                co = es["cof"][t][r]
                                for k in range(kt):
                                    m = mp.tile([P, P], b16, tag="m")
                                    mbuild(m[:], dw, sk, co + k)
                                    kdone += 1
                                    last = (kind == "g" and kdone == ktot)
                                    nc.tensor.matmul(
                                        pa[:],
                                        lhsT=gbuf[:, (co - base + k) * P:
                                                  (co - base + k + 1) * P],
                                        rhs=m[:], start=first, stop=last)
                                    first = False
                            if kind == "s":
                                # fold lin_r into the same accumulation
                                if l == 1:
                                    nc.tensor.matmul(
                                        pa[:], lhsT=wbt("sWr1"),
                                        rhs=x0T[:, sl],
                                        start=False, stop=True)
                                elif l == 2:
                                    nc.tensor.matmul(
                                        pa[:], lhsT=wbt(f"sWr{l}", 0),
                                        rhs=x1aT[:, sl],
                                        start=False, stop=False)
                                    nc.tensor.matmul(
                                        pa[:], lhsT=wbt(f"sWr{l}", 1),
                                        rhs=x1bT[:, sl],
                                        start=False, stop=True)
                                else:
                                    # lin_r(out3) precomputed at l=2 boundary
                                    nc.tensor.matmul(
                                        pa[:], lhsT=ident_s[:],
                                        rhs=lr3T[:, sl],
                                        start=False, stop=True)
                                o = (x1aT[:, sl] if l == 1 else
                                     op.tile([P, P], b16, tag="oc",
                                             name="oc")[:])
                                # o = (pa + s_bl) + x0
                                nc.vector.scalar_tensor_tensor(
                                    out=o, in0=pa[:], scalar=wfc(f"s_bl{l}"),
                                    in1=x0T[:, sl], op0=Alu.add, op1=Alu.add)
                                if l > 1:
                                    nc.vector.tensor_tensor(
                                        out=o, in0=o, in1=x1aT[:, sl],
                                        op=Alu.add)
                                outs["s"] = o
                            else:
                                o = (x1bT[:, sl] if l == 1 else
                                     op.tile([P, P], b16, tag="og",
                                             name="og")[:])
                                nc.vector.scalar_tensor_tensor(
                                    out=o, in0=pa[:], scalar=wfc(f"g_b{l}"),
                                    in1=x0T[:, sl], op0=Alu.add, op1=Alu.add)
                                if l > 1:
                                    nc.vector.tensor_tensor(
                                        out=o, in0=o, in1=x1bT[:, sl],
                                        op=Alu.add)
                                outs["g"] = o
                        # head on out_{l+1}
                        hname = f"h{l + 1}"
                        phd = php.tile([P, 1], f32, tag="ph")
                        nc.tensor.matmul(phd[:], lhsT=outs["s"],
                                         rhs=hcol(hname, 0),
                                         start=True, stop=False)
                        nc.tensor.matmul(phd[:], lhsT=outs["g"],
                                         rhs=hcol(hname, 1),
                                         start=False, stop=True)
                        nc.vector.tensor_tensor(out=resb[:, t:t + 1],
                                                in0=resb[:, t:t + 1],
                                                in1=phd[:], op=Alu.add)
                        if l == 3:
                            continue
                        # table rows for layer l+1
                        ln = l + 1
                        ps = pp.tile([P, P], f32, tag="ps")
                        nc.tensor.matmul(ps[:], lhsT=outs["s"],
                                         rhs=wbt(f"sWl{ln}", 0),
                                         start=True, stop=False)
                        nc.tensor.matmul(ps[:], lhsT=outs["g"],
                                         rhs=wbt(f"sWl{ln}", 1),
                                         start=False, stop=True)
                        pg = pp.tile([P, P], f32, tag="pg")
                        nc.tensor.matmul(pg[:], lhsT=outs["s"],
                                         rhs=wbt(f"gW{ln}", 0),
                                         start=True, stop=False)
                        nc.tensor.matmul(pg[:], lhsT=outs["g"],
                                         rhs=wbt(f"gW{ln}", 1),
                                         start=False, stop=True)
                        nc.scalar.activation(
                            tbs[:, (2 * j) * P:(2 * j + 1) * P],
                            ps[:], Act.Copy)
                        nc.scalar.activation(
                            tbs[:, (2 * j + 1) * P:(2 * j + 2) * P],
                            pg[:], Act.Copy, scale=rso_s[:, t:t + 1])
                        if l == 2:
                            # lin_r for layer 3: out3 @ s3_Wr, stored bf16
                            pl3 = pp.tile([P, P], f32, tag="ps")
                            nc.tensor.matmul(pl3[:], lhsT=wbt("sWr3", 0),
                                             rhs=outs["s"],
                                             start=True, stop=False)
                            nc.tensor.matmul(pl3[:], lhsT=wbt("sWr3", 1),
                                             rhs=outs["g"],
                                             start=False, stop=True)
                            nc.scalar.activation(lr3T[:, sl], pl3[:],
                                                 Act.Copy)
                    if l < 3:
                        nc.sync.dma_start(
                            out=sh_next[t0 * P:(t0 + ntl) * P, :]
                            .rearrange("(j p) c -> p j c", p=P),
                            in_=tbs[:].rearrange("p (j c) -> p j c", c=2 * P))
                if l < 3:
                    if not int(os.environ.get("GNN_NOBAR", "1")):
                        tc.strict_bb_all_engine_barrier()
                    nc.gpsimd.collective_compute(
                        "AllGather", mybir.AluOpType.bypass,
                        replica_groups=[list(range(NCORES))],
                        ins=[sh_next[:]], outs=[tbls[l][:]])
                    if not int(os.environ.get("GNN_NOBAR2", "0")):
                        tc.strict_bb_all_engine_barrier()

            # ---- output ----
            nc.vector.tensor_scalar(resb[:], resb[:],
                                    float(meta["total_bias"]), None,
                                    op0=Alu.add)
            nc.sync.dma_start(out=res_d[:], in_=resb[:])
        _stack.close()

    nc.compile()
    return nc


# ----------------------------------------------------------------------------
# entry point (same harness as v1)
# ----------------------------------------------------------------------------

def _run_and_bench(nc, in_maps, iters):
    import time
    import jax
    import numpy as np
    from jax.sharding import Mesh, PartitionSpec
    from jax.experimental.shard_map import shard_map
    import concourse.mybir as mybir
    from concourse import bass2jax

    bass2jax.install_neuronx_cc_hook()
    partition_name = (nc.partition_id_tensor.name
                      if nc.partition_id_tensor else None)
    in_names, out_names, out_avals, zero_outs = [], [], [], []
    for alloc in nc.m.functions[0].allocations:
        if not isinstance(alloc, mybir.MemoryLocationSet):
            continue
        name = alloc.memorylocations[0].name
        if alloc.kind == "ExternalInput":
            if name != partition_name:
                in_names.append(name)
        elif alloc.kind == "ExternalOutput":
            shape = tuple(alloc.tensor_shape)
            dtype = mybir.dt.np(alloc.dtype)
            out_names.append(name)
            out_avals.append(jax.core.ShapedArray(shape, dtype))
            zero_outs.append(np.zeros(shape, dtype))
    n_params = len(in_names)
    all_in_names = list(in_names) + out_names
    if partition_name is not None:
        all_in_names.append(partition_name)

    def _body(*args):
        operands = list(args)
        if partition_name is not None:
            operands.append(bass2jax.partition_id_tensor())
        outs = bass2jax._bass_exec_p.bind(
            *operands, out_avals=tuple(out_avals),
            in_names=tuple(all_in_names), out_names=tuple(out_names),
            lowering_input_output_aliases=(),
            sim_require_finite=True, sim_require_nnan=True, nc=nc)
        return tuple(outs)

    devices = jax.devices()[:NCORES]
    mesh = Mesh(np.asarray(devices), ("core",))
    in_specs = (PartitionSpec("core"),) * (n_params + len(out_names))
    out_specs = (PartitionSpec("core"),) * len(out_names)
    sharded = jax.jit(shard_map(_body, mesh=mesh, in_specs=in_specs,
                                out_specs=out_specs, check_rep=False),
                      keep_unused=True)
    concat_in = [
        np.concatenate([np.asarray(in_maps[c][nm]) for c in range(NCORES)], 0)
        for nm in in_names]
    concat_zeros = [np.zeros((NCORES * z.shape[0], *z.shape[1:]), z.dtype)
                    for z in zero_outs]
    out_arrs = sharded(*concat_in, *concat_zeros)
    jax.block_until_ready(out_arrs)

    per_exec_ns = None
    if iters > 0:
        from jax.sharding import NamedSharding
        dev_in = [jax.device_put(a, NamedSharding(mesh, PartitionSpec("core")))
                  for a in concat_in]
        dev_zero = [jax.device_put(z, NamedSharding(mesh, PartitionSpec("core")))
                    for z in concat_zeros]
        r = sharded(*dev_in, *dev_zero)
        jax.block_until_ready(r)
        t1 = time.perf_counter()
        rs_ = [sharded(*dev_in, *dev_zero) for _ in range(iters)]
        jax.block_until_ready(rs_)
        t2 = time.perf_counter()
        per_exec_ns = (t2 - t1) / iters * 1e9

    results = [
        {nm: np.asarray(out_arrs[i]).reshape(NCORES, *out_avals[i].shape)[c]
         for i, nm in enumerate(out_names)}
        for c in range(NCORES)]
    return results, per_exec_ns


def kernel(**inputs):
    global LAST_EXEC_NS, LAST_TRACE

    meta, in_maps = _prep(inputs)
    nc = _build(meta)

    iters = int(os.environ.get("GNN_BENCH", "0"))
    results, per_exec_ns = _run_and_bench(nc, in_maps, iters)
    LAST_EXEC_NS = per_exec_ns
    LAST_TRACE = None

    out = np.empty((N, 1), np.float32)
    for c in range(NCORES):
        r = results[c]["res"]  # [128, NT]
        out[c * SHARD:(c + 1) * SHARD, 0] = r.T.reshape(-1)[:SHARD]
    return out
